# revision 1
# baseline (speedup 1.0000x reference)
"""BitLinear (ternary-weight / int8-activation quantized linear) on 8 TRN2 NeuronCores.

Computation (matches reference):
    w_scale = mean(|W|, axis=in) + eps            # [out, 1]
    w_quant = clip(round(W / w_scale), -1, 1)     # ternary
    a_scale = max(|x|, axis=in) + eps             # per token
    a_quant = round(x / a_scale * 127)            # int8 range
    y       = (a_quant @ (w_quant * alpha).T) * w_scale * a_scale / 127

Key numerics: a_quant in [-127,127] and w_quant in {-1,0,1} are exactly
representable in bf16; products are integers <= 127 and row sums < 2^24, so a
bf16 PE matmul with fp32 PSUM accumulation is bit-exact. Rounding to nearest
even is implemented with the (v + 1.5*2^23) - 1.5*2^23 trick in fp32.

Sharding: 2 token groups x 4 out_feature groups across 8 cores.  Per core:
x [4096, 2048], w [2048, 2048], alpha [2048], out [4096, 2048].
All tensors are loaded in natural (contiguous) layout; the transposes needed to
put the contraction dim on partitions are done on the PE as identity matmuls on
the quantized bf16 data (fast regular matmuls, not slow transpose-mode ones).
"""

import numpy as np

P = 128
K = 2048          # in_features (contraction)
TOK = 8192        # total tokens (2*4096)
OUT = 8192        # total out_features
TG, OG = 2, 4     # token groups x out groups = 8 cores
T_LOC = TOK // TG   # 4096 tokens per core
O_LOC = OUT // OG   # 2048 out_features per core
KT = K // P         # 16 contraction chunks
NBLK = T_LOC // P   # 32 token blocks per core
WT = O_LOC // P     # 16 weight tiles per core
NSL = O_LOC // 512  # 4 psum n-slices
EPS = 1e-8
MAGIC = 12582912.0  # 1.5 * 2^23; add/sub rounds fp32 to nearest-even integer

_CACHE: dict = {}


def _build_nc():
    import concourse.bacc as bacc
    import concourse.mybir as mybir
    from concourse.tile import TileContext
    from concourse.masks import make_identity

    f32 = mybir.dt.float32
    bf16 = mybir.dt.bfloat16
    ALU = mybir.AluOpType
    ACTF = mybir.ActivationFunctionType
    AX = mybir.AxisListType

    nc = bacc.Bacc("TRN2", target_bir_lowering=False, debug=False, num_devices=8)
    x_d = nc.dram_tensor("x", [T_LOC, K], f32, kind="ExternalInput").ap()
    w_d = nc.dram_tensor("w", [O_LOC, K], f32, kind="ExternalInput").ap()
    al_d = nc.dram_tensor("alpha", [1, O_LOC], f32, kind="ExternalInput").ap()
    y_d = nc.dram_tensor("y", [T_LOC, O_LOC], f32, kind="ExternalOutput").ap()

    with TileContext(nc) as tc:
        with (
            tc.tile_pool(name="singles", bufs=1) as singles,
            tc.tile_pool(name="wpool", bufs=2) as wpool,
            tc.tile_pool(name="wsmall", bufs=2) as wsmall,
            tc.tile_pool(name="wqpool", bufs=2) as wqpool,
            tc.tile_pool(name="xpool", bufs=3) as xpool,
            tc.tile_pool(name="qsmall", bufs=3) as qsmall,
            tc.tile_pool(name="tapool", bufs=2) as tapool,
            tc.tile_pool(name="aqpool", bufs=2) as aqpool,
            tc.tile_pool(name="ypool", bufs=2) as ypool,
            tc.tile_pool(name="tppool", bufs=2, space="PSUM") as tppool,
            tc.tile_pool(name="yppool", bufs=4, space="PSUM") as yppool,
        ):
            ident_bf = singles.tile([P, P], bf16)
            make_identity(nc, ident_bf)
            ident_f32 = singles.tile([P, P], f32)
            make_identity(nc, ident_f32)

            # resident: transposed ternary weights [k-part, k-chunk, out]
            w_qT = singles.tile([P, KT, O_LOC], bf16)
            so_bcast = singles.tile([P, O_LOC], f32)
            so_row = singles.tile([1, O_LOC], f32)
            alpha_row = singles.tile([1, O_LOC], f32)
            nc.sync.dma_start(alpha_row, al_d)

            # ---------- Phase A: weight quantization + transpose ----------
            for i in range(WT):
                w_tile = wpool.tile([P, K], f32, tag="w_tile", name="w_tile")
                nc.sync.dma_start(w_tile, w_d[i * P : (i + 1) * P, :])
                # two-stage |W| row sum (closer to jnp pairwise summation)
                r1 = wsmall.tile([P, KT], f32, tag="r1", name="r1")
                nc.vector.tensor_reduce(
                    out=r1,
                    in_=w_tile.rearrange("p (a b) -> p a b", b=P),
                    axis=AX.X,
                    op=ALU.add,
                    apply_absolute_value=True,
                )
                ws = wsmall.tile([P, 1], f32, tag="ws", name="ws")
                nc.vector.tensor_reduce(out=ws, in_=r1, axis=AX.X, op=ALU.add)
                # ws = sum/K + eps  (1/K is a power of two -> same as mean)
                nc.vector.tensor_scalar(
                    out=ws, in0=ws, scalar1=1.0 / K, scalar2=EPS,
                    op0=ALU.mult, op1=ALU.add,
                )
                inv_ws = wsmall.tile([P, 1], f32, tag="inv_ws", name="inv_ws")
                nc.vector.reciprocal(inv_ws, ws)
                # ws row entry for the rescale (transpose [P,1] -> [1,P] on PE)
                tpr = tppool.tile([P, 4, P], f32, tag="tp", name="tpr")
                nc.tensor.matmul(
                    tpr[0:1, 0, :], lhsT=ws, rhs=ident_f32, start=True, stop=True
                )
                nc.vector.tensor_copy(so_row[0:1, i * P : (i + 1) * P], tpr[0:1, 0, :])
                # t1 = W*inv_ws + MAGIC   (rounds to nearest-even integer)
                t1 = wpool.tile([P, K], f32, tag="t1", name="t1")
                nc.vector.tensor_scalar(
                    out=t1, in0=w_tile, scalar1=inv_ws, scalar2=MAGIC,
                    op0=ALU.mult, op1=ALU.add,
                )
                # t1 - MAGIC (exact), on ACT
                nc.scalar.activation(t1, t1, ACTF.Copy, bias=-MAGIC, scale=1.0)
                # clip to [-1, 1], cast bf16, on POOL
                wq = wqpool.tile([P, K], bf16, tag="wq", name="wq")
                nc.gpsimd.tensor_scalar(
                    out=wq, in0=t1, scalar1=1.0, scalar2=-1.0,
                    op0=ALU.min, op1=ALU.max,
                )
                # transpose 16 [128,128] chunks via identity matmul
                for g in range(4):
                    tp = tppool.tile([P, 4, P], f32, tag="tp", name="tp")
                    for jj in range(4):
                        j = 4 * g + jj
                        nc.tensor.matmul(
                            tp[:, jj, :],
                            lhsT=wq[:, j * P : (j + 1) * P],
                            rhs=ident_bf,
                            start=True, stop=True,
                        )
                    dst = w_qT[:, 4 * g : 4 * g + 4, i * P : (i + 1) * P]
                    if g % 2 == 0:
                        nc.vector.tensor_copy(dst, tp)
                    else:
                        nc.scalar.copy(dst, tp)

            # s_o row = ws * alpha, broadcast to all partitions
            nc.vector.tensor_tensor(
                out=so_row, in0=so_row, in1=alpha_row, op=ALU.mult
            )
            nc.gpsimd.partition_broadcast(so_bcast, so_row)

            # ---------- Phase B: token blocks ----------
            for b in range(NBLK):
                x_tile = xpool.tile([P, K], f32, tag="x_tile", name="x_tile")
                nc.sync.dma_start(x_tile, x_d[b * P : (b + 1) * P, :])
                amax = qsmall.tile([P, 1], f32, tag="amax", name="amax")
                nc.vector.tensor_reduce(
                    out=amax, in_=x_tile, axis=AX.X, op=ALU.max,
                    apply_absolute_value=True,
                )
                ascale = qsmall.tile([P, 1], f32, tag="ascale", name="ascale")
                nc.vector.tensor_scalar_add(ascale, amax, EPS)
                inv = qsmall.tile([P, 1], f32, tag="inv", name="inv")
                nc.vector.reciprocal(inv, ascale)
                inv127 = qsmall.tile([P, 1], f32, tag="inv127", name="inv127")
                nc.vector.tensor_scalar_mul(inv127, inv, 127.0)
                s_t = qsmall.tile([P, 1], f32, tag="s_t", name="s_t")
                nc.vector.tensor_scalar_mul(s_t, ascale, 1.0 / 127.0)
                # t_a = x*inv127 + MAGIC  (DVE, fp32 2x)
                t_a = tapool.tile([P, K], f32, tag="t_a", name="t_a")
                nc.vector.tensor_scalar(
                    out=t_a, in0=x_tile, scalar1=inv127, scalar2=MAGIC,
                    op0=ALU.mult, op1=ALU.add,
                )
                # a_q = t_a - MAGIC (exact), cast bf16, on POOL
                a_q = aqpool.tile([P, K], bf16, tag="a_q", name="a_q")
                nc.gpsimd.tensor_scalar_add(a_q, t_a, -MAGIC)
                # transpose a_q via PE identity matmuls
                a_qT = aqpool.tile([P, KT, P], bf16, tag="a_qT", name="a_qT")
                for g in range(4):
                    tp = tppool.tile([P, 4, P], f32, tag="tp", name="tp2")
                    for jj in range(4):
                        j = 4 * g + jj
                        nc.tensor.matmul(
                            tp[:, jj, :],
                            lhsT=a_q[:, j * P : (j + 1) * P],
                            rhs=ident_bf,
                            start=True, stop=True,
                        )
                    dst = a_qT[:, 4 * g : 4 * g + 4, :]
                    if g % 2 == 0:
                        nc.scalar.copy(dst, tp)
                    else:
                        nc.vector.tensor_copy(dst, tp)
                # GEMM: y[tok, o] accumulated over 16 k-chunks, 512-wide slices
                y_sb = ypool.tile([P, O_LOC], f32, tag="y_sb", name="y_sb")
                for n in range(NSL):
                    yp = yppool.tile([P, 512], f32, tag="yp", name="yp")
                    for j in range(KT):
                        nc.tensor.matmul(
                            yp,
                            lhsT=a_qT[:, j, :],
                            rhs=w_qT[:, j, n * 512 : (n + 1) * 512],
                            start=(j == 0),
                            stop=(j == KT - 1),
                        )
                    ysl = y_sb[:, n * 512 : (n + 1) * 512]
                    # y * (a_scale/127)  per-token (partition) scale, PSUM->SBUF
                    nc.scalar.activation(ysl, yp, ACTF.Copy, bias=0.0, scale=s_t)
                    # y * (w_scale*alpha) per-out (free) scale
                    nc.vector.tensor_tensor(
                        out=ysl, in0=ysl,
                        in1=so_bcast[:, n * 512 : (n + 1) * 512],
                        op=ALU.mult,
                    )
                nc.sync.dma_start(y_d[b * P : (b + 1) * P, :], y_sb)

    nc.compile()
    return nc


def _get_nc():
    if "nc" not in _CACHE:
        _CACHE["nc"] = _build_nc()
    return _CACHE["nc"]


def make_in_maps(x, weight, alpha):
    x = np.ascontiguousarray(np.asarray(x, dtype=np.float32).reshape(TOK, K))
    w = np.ascontiguousarray(np.asarray(weight, dtype=np.float32))
    al = np.ascontiguousarray(np.asarray(alpha, dtype=np.float32))
    in_maps = []
    for c in range(TG * OG):
        tg, og = divmod(c, OG)
        in_maps.append(
            {
                "x": np.ascontiguousarray(x[tg * T_LOC : (tg + 1) * T_LOC]),
                "w": np.ascontiguousarray(w[og * O_LOC : (og + 1) * O_LOC]),
                "alpha": np.ascontiguousarray(
                    al[og * O_LOC : (og + 1) * O_LOC].reshape(1, O_LOC)
                ),
            }
        )
    return in_maps


def assemble(results):
    out = np.empty((TOK, OUT), dtype=np.float32)
    for c in range(TG * OG):
        tg, og = divmod(c, OG)
        out[tg * T_LOC : (tg + 1) * T_LOC, og * O_LOC : (og + 1) * O_LOC] = results[
            c
        ]["y"]
    return out.reshape(TG, T_LOC, OUT)


def kernel(x, weight, alpha, _trace=False, **_trace_kwargs):
    from concourse.bass_utils import run_bass_kernel_spmd

    nc = _get_nc()
    in_maps = make_in_maps(x, weight, alpha)
    res = run_bass_kernel_spmd(
        nc, in_maps, core_ids=list(range(TG * OG)), trace=_trace, **_trace_kwargs
    )
    _CACHE["last_results"] = res
    return assemble(res.results)


# revision 3
# speedup vs baseline: 2.1243x; 2.1243x over previous
"""BitLinear (ternary-weight / int8-activation quantized linear) on 8 TRN2 NeuronCores.

Computation (matches reference):
    w_scale = mean(|W|, axis=in) + eps            # [out, 1]
    w_quant = clip(round(W / w_scale), -1, 1)     # ternary
    a_scale = max(|x|, axis=in) + eps             # per token
    a_quant = round(x / a_scale * 127)            # int8 range
    y       = (a_quant @ (w_quant * alpha).T) * w_scale * a_scale / 127

Key numerics: a_quant in [-127,127] and w_quant in {-1,0,1} are exactly
representable in bf16; products are integers <= 127 and row sums < 2^24, so a
bf16 PE matmul with fp32 PSUM accumulation is bit-exact. Rounding to nearest
even is implemented with the (v + 1.5*2^23) - 1.5*2^23 trick in fp32.

Sharding: 2 token groups x 4 out_feature groups across 8 cores.  Per core:
x [4096, 2048], w [2048, 2048], alpha [2048], out [4096, 2048].
All tensors are loaded in natural (contiguous) layout; the transposes needed to
put the contraction dim on partitions are done on the PE as identity matmuls on
the quantized bf16 data (fast regular matmuls, not slow transpose-mode ones).
"""

import numpy as np

P = 128
K = 2048          # in_features (contraction)
TOK = 8192        # total tokens (2*4096)
OUT = 8192        # total out_features
TG, OG = 2, 4     # token groups x out groups = 8 cores
T_LOC = TOK // TG   # 4096 tokens per core
O_LOC = OUT // OG   # 2048 out_features per core
KT = K // P         # 16 contraction chunks
NBLK = T_LOC // P   # 32 token blocks per core
WT = O_LOC // P     # 16 weight tiles per core
NSL = O_LOC // 512  # 4 psum n-slices
EPS = 1e-8
MAGIC = 12582912.0  # 1.5 * 2^23; add/sub rounds fp32 to nearest-even integer

_CACHE: dict = {}


def _build_nc():
    import concourse.bacc as bacc
    import concourse.mybir as mybir
    from concourse.tile import TileContext
    from concourse.masks import make_identity

    f32 = mybir.dt.float32
    bf16 = mybir.dt.bfloat16
    ALU = mybir.AluOpType
    ACTF = mybir.ActivationFunctionType
    AX = mybir.AxisListType

    nc = bacc.Bacc("TRN2", target_bir_lowering=False, debug=False, num_devices=8)
    x_d = nc.dram_tensor("x", [T_LOC, K], f32, kind="ExternalInput").ap()
    w_d = nc.dram_tensor("w", [O_LOC, K], f32, kind="ExternalInput").ap()
    al_d = nc.dram_tensor("alpha", [1, O_LOC], f32, kind="ExternalInput").ap()
    y_d = nc.dram_tensor("y", [T_LOC, O_LOC], f32, kind="ExternalOutput").ap()

    with TileContext(nc) as tc:
        with (
            tc.tile_pool(name="singles", bufs=1) as singles,
            tc.tile_pool(name="wpool", bufs=2) as wpool,
            tc.tile_pool(name="wsmall", bufs=2) as wsmall,
            tc.tile_pool(name="wqpool", bufs=2) as wqpool,
            tc.tile_pool(name="xpool", bufs=3) as xpool,
            tc.tile_pool(name="qsmall", bufs=3) as qsmall,
            tc.tile_pool(name="tapool", bufs=2) as tapool,
            tc.tile_pool(name="aqpool", bufs=2) as aqpool,
            tc.tile_pool(name="ypool", bufs=2) as ypool,
            tc.tile_pool(name="tppool", bufs=2, space="PSUM") as tppool,
            tc.tile_pool(name="yppool", bufs=4, space="PSUM") as yppool,
        ):
            ident_bf = singles.tile([P, P], bf16)
            make_identity(nc, ident_bf)
            ident_f32 = singles.tile([P, P], f32)
            make_identity(nc, ident_f32)

            # resident: transposed ternary weights [k-part, k-chunk, out]
            w_qT = singles.tile([P, KT, O_LOC], bf16)
            so_bcast = singles.tile([P, O_LOC], f32)
            so_row = singles.tile([1, O_LOC], f32)
            alpha_row = singles.tile([1, O_LOC], f32)
            nc.sync.dma_start(alpha_row, al_d)

            # ---------- Phase A: weight quantization + transpose ----------
            for i in range(WT):
                w_tile = wpool.tile([P, K], f32, tag="w_tile", name="w_tile")
                nc.sync.dma_start(w_tile, w_d[i * P : (i + 1) * P, :])
                # two-stage |W| row sum (closer to jnp pairwise summation)
                r1 = wsmall.tile([P, KT], f32, tag="r1", name="r1")
                nc.vector.tensor_reduce(
                    out=r1,
                    in_=w_tile.rearrange("p (a b) -> p a b", b=P),
                    axis=AX.X,
                    op=ALU.add,
                    apply_absolute_value=True,
                )
                ws = wsmall.tile([P, 1], f32, tag="ws", name="ws")
                nc.vector.tensor_reduce(out=ws, in_=r1, axis=AX.X, op=ALU.add)
                # ws = sum/K + eps  (1/K is a power of two -> same as mean)
                nc.vector.tensor_scalar(
                    out=ws, in0=ws, scalar1=1.0 / K, scalar2=EPS,
                    op0=ALU.mult, op1=ALU.add,
                )
                inv_ws = wsmall.tile([P, 1], f32, tag="inv_ws", name="inv_ws")
                nc.vector.reciprocal(inv_ws, ws)
                # ws row entry for the rescale (transpose [P,1] -> [1,P] on PE)
                tpr = tppool.tile([P, 4, P], f32, tag="tp", name="tpr")
                nc.tensor.matmul(
                    tpr[0:1, 0, :], lhsT=ws, rhs=ident_f32, start=True, stop=True
                )
                nc.vector.tensor_copy(so_row[0:1, i * P : (i + 1) * P], tpr[0:1, 0, :])
                # t1 = W*inv_ws + MAGIC   (rounds to nearest-even integer)
                t1 = wpool.tile([P, K], f32, tag="t1", name="t1")
                nc.vector.tensor_scalar(
                    out=t1, in0=w_tile, scalar1=inv_ws, scalar2=MAGIC,
                    op0=ALU.mult, op1=ALU.add,
                )
                # t1 - MAGIC (exact), on ACT
                nc.scalar.activation(t1, t1, ACTF.Copy, bias=-MAGIC, scale=1.0)
                # clip to [-1, 1], cast bf16, on DVE
                wq = wqpool.tile([P, K], bf16, tag="wq", name="wq")
                nc.vector.tensor_scalar(
                    out=wq, in0=t1, scalar1=1.0, scalar2=-1.0,
                    op0=ALU.min, op1=ALU.max,
                )
                # transpose 16 [128,128] chunks via identity matmul
                for g in range(4):
                    tp = tppool.tile([P, 4, P], f32, tag="tp", name="tp")
                    for jj in range(4):
                        j = 4 * g + jj
                        nc.tensor.matmul(
                            tp[:, jj, :],
                            lhsT=wq[:, j * P : (j + 1) * P],
                            rhs=ident_bf,
                            start=True, stop=True,
                        )
                    dst = w_qT[:, 4 * g : 4 * g + 4, i * P : (i + 1) * P]
                    if g % 2 == 0:
                        nc.vector.tensor_copy(dst, tp)
                    else:
                        nc.scalar.copy(dst, tp)

            # s_o row = ws * alpha, broadcast to all partitions
            nc.vector.tensor_tensor(
                out=so_row, in0=so_row, in1=alpha_row, op=ALU.mult
            )
            nc.gpsimd.partition_broadcast(so_bcast, so_row)

            # ---------- Phase B: token blocks ----------
            for b in range(NBLK):
                x_tile = xpool.tile([P, K], f32, tag="x_tile", name="x_tile")
                nc.sync.dma_start(x_tile, x_d[b * P : (b + 1) * P, :])
                amax = qsmall.tile([P, 1], f32, tag="amax", name="amax")
                nc.vector.tensor_reduce(
                    out=amax, in_=x_tile, axis=AX.X, op=ALU.max,
                    apply_absolute_value=True,
                )
                ascale = qsmall.tile([P, 1], f32, tag="ascale", name="ascale")
                nc.vector.tensor_scalar_add(ascale, amax, EPS)
                inv = qsmall.tile([P, 1], f32, tag="inv", name="inv")
                nc.vector.reciprocal(inv, ascale)
                inv127 = qsmall.tile([P, 1], f32, tag="inv127", name="inv127")
                nc.vector.tensor_scalar_mul(inv127, inv, 127.0)
                s_t = qsmall.tile([P, 1], f32, tag="s_t", name="s_t")
                nc.vector.tensor_scalar_mul(s_t, ascale, 1.0 / 127.0)
                # t_a = x*inv127 + MAGIC  (DVE, fp32 2x)
                t_a = tapool.tile([P, K], f32, tag="t_a", name="t_a")
                nc.vector.tensor_scalar(
                    out=t_a, in0=x_tile, scalar1=inv127, scalar2=MAGIC,
                    op0=ALU.mult, op1=ALU.add,
                )
                # a_q = t_a - MAGIC (exact), cast bf16, on ACT
                a_q = aqpool.tile([P, K], bf16, tag="a_q", name="a_q")
                nc.scalar.activation(a_q, t_a, ACTF.Copy, bias=-MAGIC, scale=1.0)
                # transpose a_q via PE identity matmuls
                a_qT = aqpool.tile([P, KT, P], bf16, tag="a_qT", name="a_qT")
                for g in range(4):
                    tp = tppool.tile([P, 4, P], f32, tag="tp", name="tp2")
                    for jj in range(4):
                        j = 4 * g + jj
                        nc.tensor.matmul(
                            tp[:, jj, :],
                            lhsT=a_q[:, j * P : (j + 1) * P],
                            rhs=ident_bf,
                            start=True, stop=True,
                        )
                    dst = a_qT[:, 4 * g : 4 * g + 4, :]
                    if g % 2 == 0:
                        nc.scalar.copy(dst, tp)
                    else:
                        nc.vector.tensor_copy(dst, tp)
                # GEMM: y[tok, o] accumulated over 16 k-chunks, 512-wide slices
                y_sb = ypool.tile([P, O_LOC], f32, tag="y_sb", name="y_sb")
                for n in range(NSL):
                    yp = yppool.tile([P, 512], f32, tag="yp", name="yp")
                    for j in range(KT):
                        nc.tensor.matmul(
                            yp,
                            lhsT=a_qT[:, j, :],
                            rhs=w_qT[:, j, n * 512 : (n + 1) * 512],
                            start=(j == 0),
                            stop=(j == KT - 1),
                        )
                    ysl = y_sb[:, n * 512 : (n + 1) * 512]
                    # y * (a_scale/127)  per-token (partition) scale, PSUM->SBUF
                    nc.scalar.activation(ysl, yp, ACTF.Copy, bias=0.0, scale=s_t)
                    # y * (w_scale*alpha) per-out (free) scale
                    nc.vector.tensor_tensor(
                        out=ysl, in0=ysl,
                        in1=so_bcast[:, n * 512 : (n + 1) * 512],
                        op=ALU.mult,
                    )
                nc.sync.dma_start(y_d[b * P : (b + 1) * P, :], y_sb)

    nc.compile()
    return nc


def _get_nc():
    if "nc" not in _CACHE:
        _CACHE["nc"] = _build_nc()
    return _CACHE["nc"]


def make_in_maps(x, weight, alpha):
    x = np.ascontiguousarray(np.asarray(x, dtype=np.float32).reshape(TOK, K))
    w = np.ascontiguousarray(np.asarray(weight, dtype=np.float32))
    al = np.ascontiguousarray(np.asarray(alpha, dtype=np.float32))
    in_maps = []
    for c in range(TG * OG):
        tg, og = divmod(c, OG)
        in_maps.append(
            {
                "x": np.ascontiguousarray(x[tg * T_LOC : (tg + 1) * T_LOC]),
                "w": np.ascontiguousarray(w[og * O_LOC : (og + 1) * O_LOC]),
                "alpha": np.ascontiguousarray(
                    al[og * O_LOC : (og + 1) * O_LOC].reshape(1, O_LOC)
                ),
            }
        )
    return in_maps


def assemble(results):
    out = np.empty((TOK, OUT), dtype=np.float32)
    for c in range(TG * OG):
        tg, og = divmod(c, OG)
        out[tg * T_LOC : (tg + 1) * T_LOC, og * O_LOC : (og + 1) * O_LOC] = results[
            c
        ]["y"]
    return out.reshape(TG, T_LOC, OUT)


def kernel(x, weight, alpha, _trace=False, **_trace_kwargs):
    from concourse.bass_utils import run_bass_kernel_spmd

    nc = _get_nc()
    in_maps = make_in_maps(x, weight, alpha)
    res = run_bass_kernel_spmd(
        nc, in_maps, core_ids=list(range(TG * OG)), trace=_trace, **_trace_kwargs
    )
    _CACHE["last_results"] = res
    return assemble(res.results)


# revision 4
# speedup vs baseline: 2.2176x; 1.0439x over previous
"""BitLinear (ternary-weight / int8-activation quantized linear) on 8 TRN2 NeuronCores.

Computation (matches reference):
    w_scale = mean(|W|, axis=in) + eps            # [out, 1]
    w_quant = clip(round(W / w_scale), -1, 1)     # ternary
    a_scale = max(|x|, axis=in) + eps             # per token
    a_quant = round(x / a_scale * 127)            # int8 range
    y       = (a_quant @ (w_quant * alpha).T) * w_scale * a_scale / 127

Key numerics: a_quant in [-127,127] and w_quant in {-1,0,1} are exactly
representable in bf16; products are integers <= 127 and row sums < 2^24, so a
bf16 PE matmul with fp32 PSUM accumulation is bit-exact. Rounding to nearest
even is implemented with the (v + 1.5*2^23) - 1.5*2^23 trick in fp32.

Sharding: 2 token groups x 4 out_feature groups across 8 cores.  Per core:
x [4096, 2048], w [2048, 2048], alpha [2048], out [4096, 2048].
Transposes (contraction dim onto partitions) are done by the DMA xbar
transpose on the quantized bf16 tensors.  The block loop is software
pipelined: block b+AHEAD's quantization is emitted before block b's rescale
so no engine head-of-line blocks the next block's critical path.
"""

import numpy as np

P = 128
K = 2048
TOK = 8192
OUT = 8192
TG, OG = 2, 4
T_LOC = TOK // TG   # 4096
O_LOC = OUT // OG   # 2048
KT = K // P         # 16
NBLK = T_LOC // P   # 32
WT = O_LOC // P     # 16
NSL = O_LOC // 512  # 4
EPS = 1e-8
MAGIC = 12582912.0  # 1.5 * 2^23
AHEAD = 3           # blocks quantized ahead of the GEMM/rescale pipeline

_CACHE: dict = {}


def _build_nc():
    import concourse.bacc as bacc
    import concourse.mybir as mybir
    from concourse.tile import TileContext
    from concourse.masks import make_identity

    f32 = mybir.dt.float32
    bf16 = mybir.dt.bfloat16
    ALU = mybir.AluOpType
    ACTF = mybir.ActivationFunctionType
    AX = mybir.AxisListType

    nc = bacc.Bacc("TRN2", target_bir_lowering=False, debug=False, num_devices=8)
    x_d = nc.dram_tensor("x", [T_LOC, K], f32, kind="ExternalInput").ap()
    w_d = nc.dram_tensor("w", [O_LOC, K], f32, kind="ExternalInput").ap()
    al_d = nc.dram_tensor("alpha", [1, O_LOC], f32, kind="ExternalInput").ap()
    y_d = nc.dram_tensor("y", [T_LOC, O_LOC], f32, kind="ExternalOutput").ap()

    with TileContext(nc) as tc:
        with (
            tc.tile_pool(name="singles", bufs=1) as singles,
            tc.tile_pool(name="iopool", bufs=4) as iopool,
            tc.tile_pool(name="scratch", bufs=3) as scratch,
            tc.tile_pool(name="qpool", bufs=3) as qpool,
            tc.tile_pool(name="aqtpool", bufs=AHEAD + 2) as aqtpool,
            tc.tile_pool(name="wstg", bufs=2) as wstg,
            tc.tile_pool(name="wsmall", bufs=2) as wsmall,
            tc.tile_pool(name="qsmall", bufs=AHEAD + 2) as qsmall,
            tc.tile_pool(name="ypool", bufs=2) as ypool,
            tc.tile_pool(name="wsrow_pool", bufs=1, space="PSUM") as wsrow_pool,
            tc.tile_pool(name="yppool", bufs=4, space="PSUM") as yppool,
        ):
            ident_f32 = singles.tile([P, P], f32)
            make_identity(nc, ident_f32)

            w_qT = singles.tile([P, KT, O_LOC], bf16)   # [k-part, k-chunk, out]
            so_bcast = singles.tile([P, O_LOC], f32)
            so_row = singles.tile([1, O_LOC], f32)
            alpha_row = singles.tile([1, O_LOC], f32)
            nc.sync.dma_start(alpha_row, al_d)
            wsrow_ps = wsrow_pool.tile([1, O_LOC], f32)

            def emit_w_tile(i):
                w_tile = iopool.tile([P, K], f32, tag="in_f32", name="w_tile")
                nc.sync.dma_start(w_tile, w_d[i * P : (i + 1) * P, :])
                # two-stage |W| row sum (close to jnp pairwise summation)
                r1 = wsmall.tile([P, KT], f32, tag="r1", name="r1")
                nc.vector.tensor_reduce(
                    out=r1,
                    in_=w_tile.rearrange("p (a b) -> p a b", b=P),
                    axis=AX.X,
                    op=ALU.add,
                    apply_absolute_value=True,
                )
                ws = wsmall.tile([P, 1], f32, tag="ws", name="ws")
                nc.vector.tensor_reduce(out=ws, in_=r1, axis=AX.X, op=ALU.add)
                nc.vector.tensor_scalar(
                    out=ws, in0=ws, scalar1=1.0 / K, scalar2=EPS,
                    op0=ALU.mult, op1=ALU.add,
                )
                inv_ws = wsmall.tile([P, 1], f32, tag="inv_ws", name="inv_ws")
                nc.vector.reciprocal(inv_ws, ws)
                # ws row entry for rescale: [P,1] -> [1,P] on PE (fp32)
                nc.tensor.matmul(
                    wsrow_ps[0:1, i * P : (i + 1) * P], lhsT=ws, rhs=ident_f32,
                    start=True, stop=True,
                )
                nc.vector.tensor_copy(
                    so_row[0:1, i * P : (i + 1) * P],
                    wsrow_ps[0:1, i * P : (i + 1) * P],
                )
                # t1 = W*inv_ws + MAGIC (DVE), -MAGIC (ACT), clip+bf16 (DVE)
                t1 = scratch.tile([P, K], f32, tag="scr", name="t1")
                nc.vector.tensor_scalar(
                    out=t1, in0=w_tile, scalar1=inv_ws, scalar2=MAGIC,
                    op0=ALU.mult, op1=ALU.add,
                )
                nc.scalar.activation(t1, t1, ACTF.Copy, bias=-MAGIC, scale=1.0)
                wq = qpool.tile([P, K], bf16, tag="qb", name="wq")
                nc.vector.tensor_scalar(
                    out=wq, in0=t1, scalar1=1.0, scalar2=-1.0,
                    op0=ALU.min, op1=ALU.max,
                )
                # transpose [o, k] -> [k, o] via DMA xbar into contiguous
                # staging, then copy the 128-col slice into resident w_qT (ACT)
                wTs = wstg.tile([P, KT, P], bf16, tag="wTs", name="wTs")
                nc.sync.dma_start_transpose(wTs, wq)
                nc.scalar.copy(w_qT[:, :, i * P : (i + 1) * P], wTs)

            def emit_so_slice(ni):
                # s_o[o] = ws[o]*alpha[o] for o-slice ni, broadcast to 128 parts
                sl = slice(ni * 512, (ni + 1) * 512)
                so_tmp = wsmall.tile([1, 512], f32, tag="so_tmp", name="so_tmp")
                nc.vector.tensor_tensor(
                    out=so_tmp, in0=so_row[0:1, sl], in1=alpha_row[0:1, sl],
                    op=ALU.mult,
                )
                nc.gpsimd.partition_broadcast(so_bcast[:, sl], so_tmp)

            def emit_quant(b):
                x_tile = iopool.tile([P, K], f32, tag="in_f32", name="x_tile")
                nc.sync.dma_start(x_tile, x_d[b * P : (b + 1) * P, :])
                amax = qsmall.tile([P, 1], f32, tag="amax", name="amax", bufs=3)
                nc.vector.tensor_reduce(
                    out=amax, in_=x_tile, axis=AX.X, op=ALU.max,
                    apply_absolute_value=True,
                )
                ascale = qsmall.tile([P, 1], f32, tag="ascale", name="ascale", bufs=3)
                nc.vector.tensor_scalar_add(ascale, amax, EPS)
                inv = qsmall.tile([P, 1], f32, tag="inv", name="inv", bufs=3)
                nc.vector.reciprocal(inv, ascale)
                inv127 = qsmall.tile([P, 1], f32, tag="inv127", name="inv127", bufs=3)
                nc.vector.tensor_scalar_mul(inv127, inv, 127.0)
                s_t = qsmall.tile([P, 1], f32, tag="s_t", name="s_t")
                nc.vector.tensor_scalar_mul(s_t, ascale, 1.0 / 127.0)
                t_a = scratch.tile([P, K], f32, tag="scr", name="t_a")
                nc.vector.tensor_scalar(
                    out=t_a, in0=x_tile, scalar1=inv127, scalar2=MAGIC,
                    op0=ALU.mult, op1=ALU.add,
                )
                a_q = qpool.tile([P, K], bf16, tag="qb", name="a_q")
                nc.scalar.activation(a_q, t_a, ACTF.Copy, bias=-MAGIC, scale=1.0)
                a_qT = aqtpool.tile([P, KT, P], bf16, tag="a_qT", name="a_qT")
                nc.sync.dma_start_transpose(a_qT, a_q)
                return a_qT, s_t

            # ---------- Phase A (+ first AHEAD block quants interleaved) ----
            blk = {}  # b -> (a_qT, s_t)
            for i in range(WT):
                emit_w_tile(i)
                if i % 4 == 3:
                    ni = i // 4
                    emit_so_slice(ni)
                    if ni < AHEAD:
                        blk[ni] = emit_quant(ni)

            # ---------- Phase B: software-pipelined block loop --------------
            for b in range(NBLK):
                a_qT, s_t = blk.pop(b)
                y_sb = ypool.tile([P, O_LOC], f32, tag="y_sb", name="y_sb")
                yps = []
                for n in range(NSL):
                    yp = yppool.tile([P, 512], f32, tag="yp", name="yp")
                    for j in range(KT):
                        nc.tensor.matmul(
                            yp,
                            lhsT=a_qT[:, j, :],
                            rhs=w_qT[:, j, n * 512 : (n + 1) * 512],
                            start=(j == 0),
                            stop=(j == KT - 1),
                        )
                    yps.append(yp)
                if b + AHEAD < NBLK:
                    blk[b + AHEAD] = emit_quant(b + AHEAD)
                for n in range(NSL):
                    ysl = y_sb[:, n * 512 : (n + 1) * 512]
                    nc.scalar.activation(
                        ysl, yps[n], ACTF.Copy, bias=0.0, scale=s_t
                    )
                    nc.vector.tensor_tensor(
                        out=ysl, in0=ysl,
                        in1=so_bcast[:, n * 512 : (n + 1) * 512],
                        op=ALU.mult,
                    )
                nc.sync.dma_start(y_d[b * P : (b + 1) * P, :], y_sb)

    nc.compile()
    return nc


def _get_nc():
    if "nc" not in _CACHE:
        _CACHE["nc"] = _build_nc()
    return _CACHE["nc"]


def make_in_maps(x, weight, alpha):
    x = np.ascontiguousarray(np.asarray(x, dtype=np.float32).reshape(TOK, K))
    w = np.ascontiguousarray(np.asarray(weight, dtype=np.float32))
    al = np.ascontiguousarray(np.asarray(alpha, dtype=np.float32))
    in_maps = []
    for c in range(TG * OG):
        tg, og = divmod(c, OG)
        in_maps.append(
            {
                "x": np.ascontiguousarray(x[tg * T_LOC : (tg + 1) * T_LOC]),
                "w": np.ascontiguousarray(w[og * O_LOC : (og + 1) * O_LOC]),
                "alpha": np.ascontiguousarray(
                    al[og * O_LOC : (og + 1) * O_LOC].reshape(1, O_LOC)
                ),
            }
        )
    return in_maps


def assemble(results):
    out = np.empty((TOK, OUT), dtype=np.float32)
    for c in range(TG * OG):
        tg, og = divmod(c, OG)
        out[tg * T_LOC : (tg + 1) * T_LOC, og * O_LOC : (og + 1) * O_LOC] = results[
            c
        ]["y"]
    return out.reshape(TG, T_LOC, OUT)


def kernel(x, weight, alpha, _trace=False, **_trace_kwargs):
    from concourse.bass_utils import run_bass_kernel_spmd

    nc = _get_nc()
    in_maps = make_in_maps(x, weight, alpha)
    res = run_bass_kernel_spmd(
        nc, in_maps, core_ids=list(range(TG * OG)), trace=_trace, **_trace_kwargs
    )
    _CACHE["last_results"] = res
    return assemble(res.results)


# revision 16
# speedup vs baseline: 2.3393x; 1.0548x over previous
"""BitLinear (ternary-weight / int8-activation quantized linear) on 8 TRN2 NeuronCores.

Computation (matches reference):
    w_scale = mean(|W|, axis=in) + eps            # [out, 1]
    w_quant = clip(round(W / w_scale), -1, 1)     # ternary
    a_scale = max(|x|, axis=in) + eps             # per token
    a_quant = round(x / a_scale * 127)            # int8 range
    y       = (a_quant @ (w_quant * alpha).T) * w_scale * a_scale / 127

Key numerics: a_quant in [-127,127] and w_quant in {-1,0,1} are exactly
representable in bf16; products are integers <= 127 and row sums < 2^24, so a
bf16 PE matmul with fp32 PSUM accumulation is bit-exact.  Rounding to
nearest-even is the (v + 1.5*2^23) - 1.5*2^23 trick in fp32.

Sharding: 2 token groups x 4 out_feature groups across 8 cores.  Per core:
x [4096, 2048], w [2048, 2048], alpha [2048], out [4096, 2048].

Schedule: weight tiles are quantized (DVE/ACT) and transposed on the
otherwise-idle PE via identity matmuls during startup; activation blocks are
quantized and PE-transposed, with the GEMM running wave-major in chunks of 4
token blocks so early waves only depend on the first weight tiles.  The block
pipeline is software-pipelined (next blocks' quantization is emitted between
a chunk's GEMM waves) and all DMAs are plain HWDGE copies on the sync ring.
"""

import numpy as np

P = 128
K = 2048
TOK = 8192
OUT = 8192
TG, OG = 2, 4
T_LOC = TOK // TG   # 4096
O_LOC = OUT // OG   # 2048
KT = K // P         # 16
NBLK = T_LOC // P   # 32
WT = O_LOC // P     # 16
NSL = O_LOC // 512  # 4
CHUNK = 4           # token blocks per GEMM wave-chunk
EPS = 1e-8
MAGIC = 12582912.0  # 1.5 * 2^23

_CACHE: dict = {}


def _build_nc():
    import concourse.bacc as bacc
    import concourse.mybir as mybir
    from concourse.tile import TileContext
    from concourse.masks import make_identity

    f32 = mybir.dt.float32
    bf16 = mybir.dt.bfloat16
    ALU = mybir.AluOpType
    ACTF = mybir.ActivationFunctionType
    AX = mybir.AxisListType

    nc = bacc.Bacc("TRN2", target_bir_lowering=False, debug=False, num_devices=8)
    x_d = nc.dram_tensor("x", [T_LOC, K], f32, kind="ExternalInput").ap()
    w_d = nc.dram_tensor("w", [O_LOC, K], f32, kind="ExternalInput").ap()
    al_d = nc.dram_tensor("alpha", [1, O_LOC], f32, kind="ExternalInput").ap()
    y_d = nc.dram_tensor("y", [T_LOC, O_LOC], f32, kind="ExternalOutput").ap()

    with TileContext(nc) as tc:
        with (
            tc.tile_pool(name="singles", bufs=1) as singles,
            tc.tile_pool(name="iopool", bufs=3) as iopool,
            tc.tile_pool(name="scratch", bufs=2) as scratch,
            tc.tile_pool(name="qpool", bufs=2) as qpool,
            tc.tile_pool(name="aqtpool", bufs=8) as aqtpool,
            tc.tile_pool(name="wsmall", bufs=2) as wsmall,
            tc.tile_pool(name="qsmall", bufs=10) as qsmall,
            tc.tile_pool(name="ypool", bufs=CHUNK) as ypool,
            tc.tile_pool(name="tppool", bufs=3, space="PSUM") as tppool,
            tc.tile_pool(name="yppool", bufs=5, space="PSUM") as yppool,
        ):
            ident_f32 = singles.tile([P, P], f32)
            make_identity(nc, ident_f32)
            ident_bf = singles.tile([P, P], bf16)
            make_identity(nc, ident_bf)

            w_qT = singles.tile([P, KT, O_LOC], bf16)   # [k-part, k-chunk, out]
            so_bcast = singles.tile([P, O_LOC], f32)
            so_row = singles.tile([1, O_LOC], f32)
            alpha_row = singles.tile([1, O_LOC], f32)
            nc.sync.dma_start(alpha_row, al_d)

            def emit_w_tile(i):
                w_tile = iopool.tile([P, K], f32, tag="in_f32", name="w_tile")
                nc.sync.dma_start(w_tile, w_d[i * P : (i + 1) * P, :])
                # two-stage |W| row sum (close to jnp pairwise summation)
                r1 = wsmall.tile([P, KT], f32, tag="r1", name="r1")
                nc.vector.tensor_reduce(
                    out=r1,
                    in_=w_tile.rearrange("p (a b) -> p a b", b=P),
                    axis=AX.X,
                    op=ALU.add,
                    apply_absolute_value=True,
                )
                ws = wsmall.tile([P, 1], f32, tag="ws", name="ws")
                nc.vector.tensor_reduce(out=ws, in_=r1, axis=AX.X, op=ALU.add)
                nc.vector.tensor_scalar(
                    out=ws, in0=ws, scalar1=1.0 / K, scalar2=EPS,
                    op0=ALU.mult, op1=ALU.add,
                )
                inv_ws = wsmall.tile([P, 1], f32, tag="inv_ws", name="inv_ws")
                nc.vector.reciprocal(inv_ws, ws)
                # ws row entry for rescale: [P,1] -> [1,P] on PE (fp32)
                tpr = tppool.tile([P, 4, P], f32, tag="tp", name="tpr")
                nc.tensor.matmul(
                    tpr[0:1, 0, :], lhsT=ws, rhs=ident_f32, start=True, stop=True
                )
                nc.vector.tensor_copy(
                    so_row[0:1, i * P : (i + 1) * P], tpr[0:1, 0, :]
                )
                # round(W/ws): t1 = W*inv_ws + MAGIC, -MAGIC (ACT); clip (DVE)
                t1 = scratch.tile([P, K], f32, tag="scr", name="t1")
                nc.scalar.activation(
                    t1, w_tile, ACTF.Copy, bias=MAGIC, scale=inv_ws
                )
                nc.scalar.activation(t1, t1, ACTF.Copy, bias=-MAGIC, scale=1.0)
                wq = qpool.tile([P, K], bf16, tag="qb", name="wq")
                nc.vector.tensor_scalar(
                    out=wq, in0=t1, scalar1=1.0, scalar2=-1.0,
                    op0=ALU.min, op1=ALU.max,
                )
                # transpose 16 [128,128] chunks on PE (idle during startup)
                for g in range(4):
                    tp = tppool.tile([P, 4, P], f32, tag="tp", name="tp")
                    for jj in range(4):
                        j = 4 * g + jj
                        nc.tensor.matmul(
                            tp[:, jj, :],
                            lhsT=wq[:, j * P : (j + 1) * P],
                            rhs=ident_bf,
                            start=True, stop=True,
                        )
                    nc.scalar.copy(w_qT[:, 4 * g : 4 * g + 4, i * P : (i + 1) * P], tp)

            def emit_so_slice(ni):
                sl = slice(ni * 512, (ni + 1) * 512)
                so_tmp = wsmall.tile([1, 512], f32, tag="so_tmp", name="so_tmp")
                nc.vector.tensor_tensor(
                    out=so_tmp, in0=so_row[0:1, sl], in1=alpha_row[0:1, sl],
                    op=ALU.mult,
                )
                nc.gpsimd.partition_broadcast(so_bcast[:, sl], so_tmp)

            def emit_quant(b):
                x_tile = iopool.tile([P, K], f32, tag="in_f32", name="x_tile")
                nc.sync.dma_start(x_tile, x_d[b * P : (b + 1) * P, :])
                amax = qsmall.tile([P, 1], f32, tag="amax", name="amax", bufs=3)
                nc.vector.tensor_reduce(
                    out=amax, in_=x_tile, axis=AX.X, op=ALU.max,
                    apply_absolute_value=True,
                )
                ascale = qsmall.tile([P, 1], f32, tag="ascale", name="ascale", bufs=3)
                nc.vector.tensor_scalar_add(ascale, amax, EPS)
                inv = qsmall.tile([P, 1], f32, tag="inv", name="inv", bufs=3)
                nc.vector.reciprocal(inv, ascale)
                inv127 = qsmall.tile([P, 1], f32, tag="inv127", name="inv127", bufs=3)
                nc.vector.tensor_scalar_mul(inv127, inv, 127.0)
                s_t = qsmall.tile([P, 1], f32, tag="s_t", name="s_t")
                nc.vector.tensor_scalar_mul(s_t, ascale, 1.0 / 127.0)
                t_a = scratch.tile([P, K], f32, tag="scr", name="t_a")
                nc.vector.tensor_scalar(
                    out=t_a, in0=x_tile, scalar1=inv127, scalar2=MAGIC,
                    op0=ALU.mult, op1=ALU.add,
                )
                a_q = qpool.tile([P, K], bf16, tag="qb", name="a_q")
                nc.scalar.activation(a_q, t_a, ACTF.Copy, bias=-MAGIC, scale=1.0)
                a_qT = aqtpool.tile([P, KT, P], bf16, tag="a_qT", name="a_qT")
                for g in range(4):
                    tp = tppool.tile([P, 4, P], f32, tag="tp", name="tpq")
                    for jj in range(4):
                        j = 4 * g + jj
                        nc.tensor.matmul(
                            tp[:, jj, :],
                            lhsT=a_q[:, j * P : (j + 1) * P],
                            rhs=ident_bf,
                            start=True, stop=True,
                        )
                    dst = a_qT[:, 4 * g : 4 * g + 4, :]
                    if g % 2 == 0:
                        nc.vector.tensor_copy(dst, tp)
                    else:
                        nc.scalar.copy(dst, tp)
                return a_qT, s_t

            # ---------- Phase A (+ chunk-0 quants interleaved) --------------
            blk = {0: emit_quant(0)}
            for i in range(WT):
                emit_w_tile(i)
                if i % 4 == 3:
                    ni = i // 4
                    emit_so_slice(ni)
                    if ni + 1 < CHUNK:
                        blk[ni + 1] = emit_quant(ni + 1)

            # ---------- Phase B: wave-major chunks of CHUNK blocks ----------
            n_chunks = NBLK // CHUNK
            for c in range(n_chunks):
                blocks = range(c * CHUNK, (c + 1) * CHUNK)
                ys = {b: ypool.tile([P, O_LOC], f32, tag="y_sb", name="y_sb")
                      for b in blocks}
                for n in range(NSL):
                    for b in blocks:
                        a_qT, s_t = blk[b]
                        yp = yppool.tile([P, 512], f32, tag="yp", name="yp")
                        for j in range(KT):
                            nc.tensor.matmul(
                                yp,
                                lhsT=a_qT[:, j, :],
                                rhs=w_qT[:, j, n * 512 : (n + 1) * 512],
                                start=(j == 0),
                                stop=(j == KT - 1),
                            )
                        ysl = ys[b][:, n * 512 : (n + 1) * 512]
                        nc.scalar.activation(
                            ysl, yp, ACTF.Copy, bias=0.0, scale=s_t
                        )
                        nc.vector.tensor_tensor(
                            out=ysl, in0=ysl,
                            in1=so_bcast[:, n * 512 : (n + 1) * 512],
                            op=ALU.mult,
                        )
                    # quantize one block of the next chunk per wave
                    nb = (c + 1) * CHUNK + n
                    if nb < NBLK:
                        blk[nb] = emit_quant(nb)
                for b in blocks:
                    del blk[b]
                    nc.sync.dma_start(y_d[b * P : (b + 1) * P, :], ys[b])

    nc.compile()
    return nc


def _get_nc():
    if "nc" not in _CACHE:
        _CACHE["nc"] = _build_nc()
    return _CACHE["nc"]


def make_in_maps(x, weight, alpha):
    x = np.ascontiguousarray(np.asarray(x, dtype=np.float32).reshape(TOK, K))
    w = np.ascontiguousarray(np.asarray(weight, dtype=np.float32))
    al = np.ascontiguousarray(np.asarray(alpha, dtype=np.float32))
    in_maps = []
    for c in range(TG * OG):
        tg, og = divmod(c, OG)
        in_maps.append(
            {
                "x": np.ascontiguousarray(x[tg * T_LOC : (tg + 1) * T_LOC]),
                "w": np.ascontiguousarray(w[og * O_LOC : (og + 1) * O_LOC]),
                "alpha": np.ascontiguousarray(
                    al[og * O_LOC : (og + 1) * O_LOC].reshape(1, O_LOC)
                ),
            }
        )
    return in_maps


def assemble(results):
    out = np.empty((TOK, OUT), dtype=np.float32)
    for c in range(TG * OG):
        tg, og = divmod(c, OG)
        out[tg * T_LOC : (tg + 1) * T_LOC, og * O_LOC : (og + 1) * O_LOC] = results[
            c
        ]["y"]
    return out.reshape(TG, T_LOC, OUT)


def kernel(x, weight, alpha, _trace=False, **_trace_kwargs):
    from concourse.bass_utils import run_bass_kernel_spmd

    nc = _get_nc()
    in_maps = make_in_maps(x, weight, alpha)
    res = run_bass_kernel_spmd(
        nc, in_maps, core_ids=list(range(TG * OG)), trace=_trace, **_trace_kwargs
    )
    _CACHE["last_results"] = res
    return assemble(res.results)


# revision 17
# speedup vs baseline: 2.3673x; 1.0120x over previous
"""BitLinear (ternary-weight / int8-activation quantized linear) on 8 TRN2 NeuronCores.

Computation (matches reference):
    w_scale = mean(|W|, axis=in) + eps            # [out, 1]
    w_quant = clip(round(W / w_scale), -1, 1)     # ternary
    a_scale = max(|x|, axis=in) + eps             # per token
    a_quant = round(x / a_scale * 127)            # int8 range
    y       = (a_quant @ (w_quant * alpha).T) * w_scale * a_scale / 127

Key numerics: a_quant in [-127,127] and w_quant in {-1,0,1} are exactly
representable in bf16; products are integers <= 127 and row sums < 2^24, so a
bf16 PE matmul with fp32 PSUM accumulation is bit-exact.  Rounding to
nearest-even is the (v + 1.5*2^23) - 1.5*2^23 trick in fp32.

Sharding: 2 token groups x 4 out_feature groups across 8 cores.  Per core:
x [4096, 2048], w [2048, 2048], alpha [2048], out [4096, 2048].

Schedule: weight tiles are quantized (DVE/ACT) and transposed on the
otherwise-idle PE via identity matmuls during startup; activation blocks are
quantized and PE-transposed, with the GEMM running wave-major in chunks of 4
token blocks so early waves only depend on the first weight tiles.  The block
pipeline is software-pipelined (next blocks' quantization is emitted between
a chunk's GEMM waves) and all DMAs are plain HWDGE copies on the sync ring.
"""

import numpy as np

P = 128
K = 2048
TOK = 8192
OUT = 8192
TG, OG = 2, 4
T_LOC = TOK // TG   # 4096
O_LOC = OUT // OG   # 2048
KT = K // P         # 16
NBLK = T_LOC // P   # 32
WT = O_LOC // P     # 16
NSL = O_LOC // 512  # 4
CHUNK = 4           # token blocks per GEMM wave-chunk
EPS = 1e-8
MAGIC = 12582912.0  # 1.5 * 2^23

_CACHE: dict = {}


def _build_nc():
    import concourse.bacc as bacc
    import concourse.mybir as mybir
    from concourse.tile import TileContext
    from concourse.masks import make_identity

    f32 = mybir.dt.float32
    bf16 = mybir.dt.bfloat16
    ALU = mybir.AluOpType
    ACTF = mybir.ActivationFunctionType
    AX = mybir.AxisListType

    nc = bacc.Bacc("TRN2", target_bir_lowering=False, debug=False, num_devices=8)
    x_d = nc.dram_tensor("x", [T_LOC, K], f32, kind="ExternalInput").ap()
    w_d = nc.dram_tensor("w", [O_LOC, K], f32, kind="ExternalInput").ap()
    al_d = nc.dram_tensor("alpha", [1, O_LOC], f32, kind="ExternalInput").ap()
    y_d = nc.dram_tensor("y", [T_LOC, O_LOC], f32, kind="ExternalOutput").ap()

    with TileContext(nc) as tc:
        with (
            tc.tile_pool(name="singles", bufs=1) as singles,
            tc.tile_pool(name="iopool", bufs=3) as iopool,
            tc.tile_pool(name="scratch", bufs=2) as scratch,
            tc.tile_pool(name="qpool", bufs=2) as qpool,
            tc.tile_pool(name="aqtpool", bufs=8) as aqtpool,
            tc.tile_pool(name="wsmall", bufs=2) as wsmall,
            tc.tile_pool(name="qsmall", bufs=10) as qsmall,
            tc.tile_pool(name="ypool", bufs=CHUNK) as ypool,
            tc.tile_pool(name="tppool", bufs=3, space="PSUM") as tppool,
            tc.tile_pool(name="yppool", bufs=5, space="PSUM") as yppool,
        ):
            ident_f32 = singles.tile([P, P], f32)
            make_identity(nc, ident_f32)
            ident_bf = singles.tile([P, P], bf16)
            make_identity(nc, ident_bf)

            # HAM warm-up: dense dummy matmuls while the PE would idle at
            # startup (un-throttles the clock gate to 2.4 GHz before real work)
            warm_rhs = singles.tile([P, 4, P], bf16)
            nc.vector.memset(warm_rhs, 0.0)

            def emit_warm(n_mm):
                for _ in range(n_mm):
                    tp = tppool.tile([P, 4, P], f32, tag="tp", name="warm")
                    nc.tensor.matmul(tp, lhsT=ident_bf, rhs=warm_rhs,
                                     start=True, stop=True)

            w_qT = singles.tile([P, KT, O_LOC], bf16)   # [k-part, k-chunk, out]
            so_bcast = singles.tile([P, O_LOC], f32)
            so_row = singles.tile([1, O_LOC], f32)
            alpha_row = singles.tile([1, O_LOC], f32)
            nc.sync.dma_start(alpha_row, al_d)

            def emit_w_tile(i):
                w_tile = iopool.tile([P, K], f32, tag="in_f32", name="w_tile")
                nc.sync.dma_start(w_tile, w_d[i * P : (i + 1) * P, :])
                # two-stage |W| row sum (close to jnp pairwise summation)
                r1 = wsmall.tile([P, KT], f32, tag="r1", name="r1")
                nc.vector.tensor_reduce(
                    out=r1,
                    in_=w_tile.rearrange("p (a b) -> p a b", b=P),
                    axis=AX.X,
                    op=ALU.add,
                    apply_absolute_value=True,
                )
                ws = wsmall.tile([P, 1], f32, tag="ws", name="ws")
                nc.vector.tensor_reduce(out=ws, in_=r1, axis=AX.X, op=ALU.add)
                nc.vector.tensor_scalar(
                    out=ws, in0=ws, scalar1=1.0 / K, scalar2=EPS,
                    op0=ALU.mult, op1=ALU.add,
                )
                inv_ws = wsmall.tile([P, 1], f32, tag="inv_ws", name="inv_ws")
                nc.vector.reciprocal(inv_ws, ws)
                # ws row entry for rescale: [P,1] -> [1,P] on PE (fp32)
                tpr = tppool.tile([P, 4, P], f32, tag="tp", name="tpr")
                nc.tensor.matmul(
                    tpr[0:1, 0, :], lhsT=ws, rhs=ident_f32, start=True, stop=True
                )
                nc.vector.tensor_copy(
                    so_row[0:1, i * P : (i + 1) * P], tpr[0:1, 0, :]
                )
                # round(W/ws): t1 = W*inv_ws + MAGIC, -MAGIC (ACT); clip (DVE)
                t1 = scratch.tile([P, K], f32, tag="scr", name="t1")
                nc.scalar.activation(
                    t1, w_tile, ACTF.Copy, bias=MAGIC, scale=inv_ws
                )
                nc.scalar.activation(t1, t1, ACTF.Copy, bias=-MAGIC, scale=1.0)
                wq = qpool.tile([P, K], bf16, tag="qb", name="wq")
                nc.vector.tensor_scalar(
                    out=wq, in0=t1, scalar1=1.0, scalar2=-1.0,
                    op0=ALU.min, op1=ALU.max,
                )
                # transpose 16 [128,128] chunks on PE (idle during startup)
                for g in range(4):
                    tp = tppool.tile([P, 4, P], f32, tag="tp", name="tp")
                    for jj in range(4):
                        j = 4 * g + jj
                        nc.tensor.matmul(
                            tp[:, jj, :],
                            lhsT=wq[:, j * P : (j + 1) * P],
                            rhs=ident_bf,
                            start=True, stop=True,
                        )
                    nc.scalar.copy(w_qT[:, 4 * g : 4 * g + 4, i * P : (i + 1) * P], tp)

            def emit_so_slice(ni):
                sl = slice(ni * 512, (ni + 1) * 512)
                so_tmp = wsmall.tile([1, 512], f32, tag="so_tmp", name="so_tmp")
                nc.vector.tensor_tensor(
                    out=so_tmp, in0=so_row[0:1, sl], in1=alpha_row[0:1, sl],
                    op=ALU.mult,
                )
                nc.gpsimd.partition_broadcast(so_bcast[:, sl], so_tmp)

            def emit_quant(b):
                x_tile = iopool.tile([P, K], f32, tag="in_f32", name="x_tile")
                nc.sync.dma_start(x_tile, x_d[b * P : (b + 1) * P, :])
                amax = qsmall.tile([P, 1], f32, tag="amax", name="amax", bufs=3)
                nc.vector.tensor_reduce(
                    out=amax, in_=x_tile, axis=AX.X, op=ALU.max,
                    apply_absolute_value=True,
                )
                ascale = qsmall.tile([P, 1], f32, tag="ascale", name="ascale", bufs=3)
                nc.vector.tensor_scalar_add(ascale, amax, EPS)
                inv = qsmall.tile([P, 1], f32, tag="inv", name="inv", bufs=3)
                nc.vector.reciprocal(inv, ascale)
                inv127 = qsmall.tile([P, 1], f32, tag="inv127", name="inv127", bufs=3)
                nc.vector.tensor_scalar_mul(inv127, inv, 127.0)
                s_t = qsmall.tile([P, 1], f32, tag="s_t", name="s_t")
                nc.vector.tensor_scalar_mul(s_t, ascale, 1.0 / 127.0)
                t_a = scratch.tile([P, K], f32, tag="scr", name="t_a")
                nc.vector.tensor_scalar(
                    out=t_a, in0=x_tile, scalar1=inv127, scalar2=MAGIC,
                    op0=ALU.mult, op1=ALU.add,
                )
                a_q = qpool.tile([P, K], bf16, tag="qb", name="a_q")
                nc.scalar.activation(a_q, t_a, ACTF.Copy, bias=-MAGIC, scale=1.0)
                a_qT = aqtpool.tile([P, KT, P], bf16, tag="a_qT", name="a_qT")
                for g in range(4):
                    tp = tppool.tile([P, 4, P], f32, tag="tp", name="tpq")
                    for jj in range(4):
                        j = 4 * g + jj
                        nc.tensor.matmul(
                            tp[:, jj, :],
                            lhsT=a_q[:, j * P : (j + 1) * P],
                            rhs=ident_bf,
                            start=True, stop=True,
                        )
                    dst = a_qT[:, 4 * g : 4 * g + 4, :]
                    if g % 2 == 0:
                        nc.vector.tensor_copy(dst, tp)
                    else:
                        nc.scalar.copy(dst, tp)
                return a_qT, s_t

            # ---------- Phase A (+ chunk-0 quants interleaved) --------------
            emit_warm(44)
            blk = {0: emit_quant(0)}
            for i in range(WT):
                emit_w_tile(i)
                emit_warm(4)
                if i % 4 == 3:
                    ni = i // 4
                    emit_so_slice(ni)
                    if ni + 1 < CHUNK:
                        blk[ni + 1] = emit_quant(ni + 1)

            # ---------- Phase B: wave-major chunks of CHUNK blocks ----------
            n_chunks = NBLK // CHUNK
            for c in range(n_chunks):
                blocks = range(c * CHUNK, (c + 1) * CHUNK)
                ys = {b: ypool.tile([P, O_LOC], f32, tag="y_sb", name="y_sb")
                      for b in blocks}
                for n in range(NSL):
                    for b in blocks:
                        a_qT, s_t = blk[b]
                        yp = yppool.tile([P, 512], f32, tag="yp", name="yp")
                        for j in range(KT):
                            nc.tensor.matmul(
                                yp,
                                lhsT=a_qT[:, j, :],
                                rhs=w_qT[:, j, n * 512 : (n + 1) * 512],
                                start=(j == 0),
                                stop=(j == KT - 1),
                            )
                        ysl = ys[b][:, n * 512 : (n + 1) * 512]
                        nc.scalar.activation(
                            ysl, yp, ACTF.Copy, bias=0.0, scale=s_t
                        )
                        nc.vector.tensor_tensor(
                            out=ysl, in0=ysl,
                            in1=so_bcast[:, n * 512 : (n + 1) * 512],
                            op=ALU.mult,
                        )
                    # quantize one block of the next chunk per wave
                    nb = (c + 1) * CHUNK + n
                    if nb < NBLK:
                        blk[nb] = emit_quant(nb)
                for b in blocks:
                    del blk[b]
                    nc.sync.dma_start(y_d[b * P : (b + 1) * P, :], ys[b])

    nc.compile()
    return nc


def _get_nc():
    if "nc" not in _CACHE:
        _CACHE["nc"] = _build_nc()
    return _CACHE["nc"]


def make_in_maps(x, weight, alpha):
    x = np.ascontiguousarray(np.asarray(x, dtype=np.float32).reshape(TOK, K))
    w = np.ascontiguousarray(np.asarray(weight, dtype=np.float32))
    al = np.ascontiguousarray(np.asarray(alpha, dtype=np.float32))
    in_maps = []
    for c in range(TG * OG):
        tg, og = divmod(c, OG)
        in_maps.append(
            {
                "x": np.ascontiguousarray(x[tg * T_LOC : (tg + 1) * T_LOC]),
                "w": np.ascontiguousarray(w[og * O_LOC : (og + 1) * O_LOC]),
                "alpha": np.ascontiguousarray(
                    al[og * O_LOC : (og + 1) * O_LOC].reshape(1, O_LOC)
                ),
            }
        )
    return in_maps


def assemble(results):
    out = np.empty((TOK, OUT), dtype=np.float32)
    for c in range(TG * OG):
        tg, og = divmod(c, OG)
        out[tg * T_LOC : (tg + 1) * T_LOC, og * O_LOC : (og + 1) * O_LOC] = results[
            c
        ]["y"]
    return out.reshape(TG, T_LOC, OUT)


def kernel(x, weight, alpha, _trace=False, **_trace_kwargs):
    from concourse.bass_utils import run_bass_kernel_spmd

    nc = _get_nc()
    in_maps = make_in_maps(x, weight, alpha)
    res = run_bass_kernel_spmd(
        nc, in_maps, core_ids=list(range(TG * OG)), trace=_trace, **_trace_kwargs
    )
    _CACHE["last_results"] = res
    return assemble(res.results)
